# revision 9
# baseline (speedup 1.0000x reference)
"""Trainium2 Bass kernel for AttnBlock (rmsnorm -> qkv -> block-causal frame
attention -> output proj -> residual).

Sharding (v2, sequence-parallel per the hint):
  * Queries: core i owns the i-th 128-token slice of every frame (8 query
    chunks of 128).  Query chunk s (frame s) attends frames 0..s, so spans are
    compile-time constants, identical on every core -> clean SPMD.
  * K/V projections: core i computes K/V ONLY for frame i (1024 tokens), then
    two AllGathers (K first, then V) distribute them in fp8e4 (transport +
    matmul dtype; scores here are tiny, sigma~0.2, so fp8 K/V error is ~1e-4).
  * While AG_K is in flight the PE does V/Q projections; while AG_V is in
    flight the PE runs slice-0 scores (lookahead, at-tiles buffered in SBUF)
    so the first AV matmul lands right as V arrives.

Matmul inputs bf16/fp8 (fp32 matmul is 4x slower); accumulation fp32 in PSUM.
Softmax skips max-subtraction (scores are O(0.2)); row sums via ones-vector
matmul on PE; 1/sum applied after the attn@V contraction (commutes with WO).
"""

import math
import os
import sys

import numpy as np

for _p in ("/opt/trn_rl_repo",):
    if _p not in sys.path:
        sys.path.insert(0, _p)

import ml_dtypes  # noqa: E402

import concourse.bass as bass  # noqa: E402
import concourse.tile as tile  # noqa: E402
from concourse import bacc  # noqa: E402
from concourse import mybir  # noqa: E402
from concourse.bass_utils import run_bass_kernel_spmd  # noqa: E402

BF16 = mybir.dt.bfloat16
FP8 = mybir.dt.float8e4
F32 = mybir.dt.float32

B, C, NF, H, W = 1, 512, 8, 32, 32
NHW = H * W          # 1024 tokens per frame
SEQ = NF * NHW       # 8192
NCORES = 8
P = 128              # partitions
CC = C // P          # 4 channel chunks
NT = NHW // P        # 8 key tiles per frame
SQRT_C = math.sqrt(C)
INV_SQRT_C = 1.0 / SQRT_C

KV_DT = FP8 if os.environ.get("BASS_KV_DTYPE", "fp8") == "fp8" else BF16

LAST_RESULTS = None  # BassKernelResults of the most recent run (for test.py)


def _ensure_axon_hooks():
    """bass_utils' trace path imports antenv.axon_hooks, which is absent from
    some container snapshots.  Provide the tiny registry (and wire the ctypes
    NTFF hook from trn_agent_boot when available) so tracing degrades
    gracefully instead of crashing."""
    import types

    try:
        import antenv.axon_hooks  # noqa: F401

        return
    except Exception:
        pass
    try:
        import antenv
    except Exception:
        antenv = types.ModuleType("antenv")
        sys.modules["antenv"] = antenv
    mod = types.ModuleType("antenv.axon_hooks")
    _h = [None]
    mod.set_axon_ntff_profile_hook = lambda hook: _h.__setitem__(0, hook)
    mod.get_axon_ntff_profile_hook = lambda: _h[0]
    sys.modules["antenv.axon_hooks"] = mod
    antenv.axon_hooks = mod
    try:
        from trn_agent_boot.trn_boot import _ntff_profile_via_ctypes

        hook = _ntff_profile_via_ctypes("/opt/axon/libaxon_pjrt.so")
        if hook is not None:
            mod.set_axon_ntff_profile_hook(hook)
    except Exception:
        pass


def _bcast(ap, p=P):
    """AP that reads a DRAM row and replicates it across p partitions."""
    return bass.AP(tensor=ap.tensor, offset=ap.offset, ap=[[0, p], *list(ap.ap)])


def _build_nc():
    nc = bacc.Bacc(num_devices=NCORES)

    xfi = nc.declare_dram_parameter("xfi", [C, NHW], BF16, isOutput=False)
    xo = nc.declare_dram_parameter("xo", [C, NHW], F32, isOutput=False)
    wqT = nc.declare_dram_parameter("wqT", [C, C], BF16, isOutput=False)
    wkT = nc.declare_dram_parameter("wkT", [C, C], BF16, isOutput=False)
    wvT = nc.declare_dram_parameter("wvT", [C, C], BF16, isOutput=False)
    woT = nc.declare_dram_parameter("woT", [C, C], BF16, isOutput=False)
    gamma = nc.declare_dram_parameter("gamma", [C], F32, isOutput=False)
    bq = nc.declare_dram_parameter("bq", [C], F32, isOutput=False)
    bk = nc.declare_dram_parameter("bk", [C], F32, isOutput=False)
    bv = nc.declare_dram_parameter("bv", [C], F32, isOutput=False)
    bo = nc.declare_dram_parameter("bo", [C], F32, isOutput=False)
    out = nc.declare_dram_parameter("out", [C, NHW], F32, isOutput=True)

    with tile.TileContext(nc) as tc:
        _emit(tc, xfi, xo, wqT, wkT, wvT, woT, gamma, bq, bk, bv, bo, out)
    return nc


def _emit(tc, xfi, xo, wqT, wkT, wvT, woT, gamma, bq, bk, bv, bo, out):
    nc = tc.nc
    Act = mybir.ActivationFunctionType
    Alu = mybir.AluOpType
    RG = [[i for i in range(NCORES)]]

    with (
        tc.tile_pool(name="dram", bufs=1, space="DRAM") as drp,
        tc.tile_pool(name="singles", bufs=1) as singles,
    ):
        # ---- DRAM scratch ----
        agw_in = drp.tile([1, 128], F32)
        agw_out = drp.tile([NCORES, 128], F32, addr_space="Shared")
        agk_in = drp.tile([C, NHW], KV_DT)
        agk_out = drp.tile([NCORES, C, NHW], KV_DT, addr_space="Shared")
        agv_in = drp.tile([NHW, C], KV_DT)
        agv_out = drp.tile([NCORES, NHW, C], KV_DT, addr_space="Shared")
        r_dram = drp.tile([2, 1, NHW], F32)    # rms scale rows (frame i, own)
        rc_dram = drp.tile([1, NHW], F32)      # softmax 1/sum row (own queries)

        # ---- constants & weights in SBUF ----
        def _wsb(wT, eng=None):
            t = singles.tile([P, CC, C], BF16, tag=f"w_{wT.name}")
            (eng or nc.sync).dma_start(t[:], wT[:].rearrange("(cc p) o -> p cc o", p=P))
            return t

        def _col(v):
            t = singles.tile([P, CC], F32, tag=f"col_{v.name}")
            nc.scalar.dma_start(t[:], v[:].rearrange("(cc p) -> p cc", p=P))
            return t

        gammaCol, bqCol, bkCol, boCol = _col(gamma), _col(bq), _col(bk), _col(bo)
        gsCol = singles.tile([P, CC], F32)          # gamma * sqrt(C)
        nc.vector.tensor_scalar_mul(gsCol[:], gammaCol[:], float(SQRT_C))

        bvB = singles.tile([P, C], F32)             # bv broadcast along partitions
        nc.scalar.dma_start(bvB[:], _bcast(bv[:]))

        warm_sb = singles.tile([1, 128], F32)
        nc.vector.memset(warm_sb[:], 0.0)
        nc.gpsimd.dma_start(agw_in[:], warm_sb[:])
        nc.gpsimd.collective_compute(
            "AllGather",
            mybir.AluOpType.bypass,
            replica_groups=[[i for i in range(NCORES)]],
            ins=[agw_in[:].opt()],
            outs=[agw_out[:].opt()],
        )
        ones_bf = singles.tile([P, 1], BF16)
        nc.vector.memset(ones_bf[:], 1.0)
        ones_row = singles.tile([1, P], F32)
        nc.vector.memset(ones_row[:], 1.0)
        eps_row = singles.tile([1, 1], F32)
        nc.vector.memset(eps_row[:], 1e-24)

        # ---- big persistent tensors ----
        K_sb = singles.tile([P, CC, SEQ], KV_DT)    # gathered keys [c_chunk, seq]
        Q_sb = singles.tile([P, CC, NHW], BF16)     # own queries
        xo_sb = singles.tile([P, CC, NHW], F32)     # own x (residual)
        at0_sb = singles.tile([P, 4 * NT, 512], BF16)  # slice-0 exp(scores) buffer

        # ================= phase A: norm + projections + gathers ============
        with (
            tc.tile_pool(name="stage", bufs=2) as stage,
            tc.tile_pool(name="hpool", bufs=2) as hpool,
            tc.tile_pool(name="rows", bufs=3) as rows,
            tc.tile_pool(name="vstage", bufs=4) as vstage,
            tc.tile_pool(name="ppA", bufs=4, space="PSUM") as ppA,
            tc.tile_pool(name="ppS", bufs=2, space="PSUM") as ppS,
            tc.tile_pool(name="ppR", bufs=2, space="PSUM") as ppR,
        ):
            # DMA priority order: xfi is the critical path to AG_K, then the
            # weights in first-use order; xo/wq/wo only matter mid-gather.
            xfi_sb = stage.tile([P, CC, NHW], BF16, tag="xfi")
            nc.sync.dma_start(xfi_sb[:], xfi[:].rearrange("(cc p) s -> p cc s", p=P))
            wk_sb, wv_sb = _wsb(wkT), _wsb(wvT)

            def _norm_half(x_in, pc):
                """rms rows for a 512-token half -> [P, 512] broadcast in PSUM
                (ones-matmul broadcast; no DRAM round-trip)."""
                x2 = stage.tile([P, CC, 512], BF16, tag="x2")
                nc.scalar.activation(x2[:], x_in[:], Act.Square)
                ps = ppS.tile([1, 512], F32)
                for cc in range(CC):
                    nc.tensor.matmul(
                        ps[:],
                        lhsT=ones_bf[:],
                        rhs=x2[:, cc, :],
                        start=(cc == 0),
                        stop=(cc == CC - 1),
                    )
                nrm = rows.tile([1, 512], F32, tag="nrm")
                nc.scalar.activation(nrm[:], ps[:], Act.Sqrt, bias=eps_row[:])
                rrow = rows.tile([1, 512], F32, tag="rrow")
                nc.vector.reciprocal_approx_fast(out=rrow[:], in_=nrm[:])
                rB = ppR.tile([P, 512], F32)
                nc.tensor.matmul(rB[:], lhsT=ones_row[:], rhs=rrow[:])
                return rB

            def _h_from(xsrc):
                ht = hpool.tile([P, CC, NHW], BF16)
                for pc in range(2):
                    rB = _norm_half(xsrc[:, :, pc * 512 : (pc + 1) * 512], pc)
                    for cc in range(CC):
                        nc.vector.scalar_tensor_tensor(
                            out=ht[:, cc, pc * 512 : (pc + 1) * 512],
                            in0=xsrc[:, cc, pc * 512 : (pc + 1) * 512],
                            scalar=gsCol[:, cc : cc + 1],
                            in1=rB[:],
                            op0=Alu.mult,
                            op1=Alu.mult,
                        )
                return ht

            h_i = _h_from(xfi_sb)

            # ---- K_i projection -> agk_in (half-granularity) -> AllGather K
            ksb_i = stage.tile([P, CC, NHW], KV_DT, tag="ksb")
            agk_in_r = agk_in[:].rearrange("(cc p) s -> p cc s", p=P)
            for pc in range(2):
                for oc in range(CC):
                    psk = ppA.tile([P, 512], F32, tag="ps", name="psk")
                    for cc in range(CC):
                        nc.tensor.matmul(
                            psk[:],
                            lhsT=wk_sb[:, cc, oc * P : (oc + 1) * P],
                            rhs=h_i[:, cc, pc * 512 : (pc + 1) * 512],
                            start=(cc == 0),
                            stop=(cc == CC - 1),
                        )
                    nc.scalar.activation(
                        ksb_i[:, oc, pc * 512 : (pc + 1) * 512],
                        psk[:],
                        Act.Identity,
                        bias=bkCol[:, oc : oc + 1],
                    )
                nc.sync.dma_start(
                    agk_in_r[:, :, pc * 512 : (pc + 1) * 512],
                    ksb_i[:, :, pc * 512 : (pc + 1) * 512],
                )
            nc.gpsimd.collective_compute(
                "AllGather",
                mybir.AluOpType.bypass,
                replica_groups=RG,
                ins=[agk_in[:].opt()],
                outs=[agk_out[:].opt()],
            )

            # ---- V_i projection -> agv_in -> AllGather V ----
            for t in range(NT):
                psv = ppA.tile([P, 512], F32, tag="ps", name="psv")
                for cc in range(CC):
                    nc.tensor.matmul(
                        psv[:],
                        lhsT=h_i[:, cc, t * P : (t + 1) * P],
                        rhs=wv_sb[:, cc, :],
                        start=(cc == 0),
                        stop=(cc == CC - 1),
                    )
                vt_s = vstage.tile([P, C], KV_DT)
                nc.vector.scalar_tensor_tensor(
                    out=vt_s[:],
                    in0=psv[:],
                    scalar=1.0,
                    in1=bvB[:],
                    op0=Alu.mult,
                    op1=Alu.add,
                )
                nc.sync.dma_start(agv_in[t * P : (t + 1) * P, :], vt_s[:])
            nc.gpsimd.collective_compute(
                "AllGather",
                mybir.AluOpType.bypass,
                replica_groups=RG,
                ins=[agv_in[:].opt()],
                outs=[agv_out[:].opt()],
            )

            # ---- own norm + h + Q projection (fills the gather window) ----
            nc.scalar.dma_start(xo_sb[:], xo[:].rearrange("(cc p) s -> p cc s", p=P))
            wq_sb, wo_sb = _wsb(wqT, nc.scalar), _wsb(woT, nc.scalar)
            xbf = stage.tile([P, CC, NHW], BF16, tag="xbf")
            nc.vector.tensor_copy(out=xbf[:], in_=xo_sb[:])
            h_own = _h_from(xbf)
            for oc in range(CC):
                for pc in range(2):
                    psq = ppA.tile([P, 512], F32, tag="ps", name="psq")
                    for cc in range(CC):
                        nc.tensor.matmul(
                            psq[:],
                            lhsT=wq_sb[:, cc, oc * P : (oc + 1) * P],
                            rhs=h_own[:, cc, pc * 512 : (pc + 1) * 512],
                            start=(cc == 0),
                            stop=(cc == CC - 1),
                        )
                    nc.scalar.activation(
                        Q_sb[:, oc, pc * 512 : (pc + 1) * 512],
                        psq[:],
                        Act.Identity,
                        bias=bqCol[:, oc : oc + 1],
                    )

            # gathered K -> SBUF, one DMA per frame (pipelines with attention;
            # emitted last so no phase-A DMA queues behind the AG_K wait)
            for j in range(NF):
                nc.sync.dma_start(
                    K_sb[:, :, j * NHW : (j + 1) * NHW],
                    agk_out[j].rearrange("(cc p) s -> p cc s", p=P),
                )

        if os.environ.get("BASS_PHASE") == "A":
            with tc.tile_pool(name="dummy", bufs=1) as dummy:
                dmy = dummy.tile([P, CC, NHW], F32)
                nc.vector.tensor_copy(out=dmy[:], in_=xo_sb[:])
                nc.sync.dma_start(out[:].rearrange("(cc p) s -> p cc s", p=P), dmy[:])
            return

        # ================= phase B: attention + output =================
        with (
            tc.tile_pool(name="atp", bufs=6) as atp,
            tc.tile_pool(name="vload", bufs=6) as vload,
            tc.tile_pool(name="onorm", bufs=2) as onorm,
            tc.tile_pool(name="rcb", bufs=2) as rcb,
            tc.tile_pool(name="rows2", bufs=2) as rows2,
            tc.tile_pool(name="outst", bufs=2) as outst,
            tc.tile_pool(name="ppSc", bufs=2, space="PSUM") as ppSc,
            tc.tile_pool(name="ppO", bufs=1, space="PSUM") as ppO,
            tc.tile_pool(name="ppSum", bufs=2, space="PSUM") as ppSum,
        ):
            def _tail(sl, rcB, onrm):
                for oc in range(CC):
                    psw = ppSc.tile([P, 512], F32, tag="sc", name="psw")
                    for cc in range(CC):
                        nc.tensor.matmul(
                            psw[:],
                            lhsT=wo_sb[:, cc, oc * P : (oc + 1) * P],
                            rhs=onrm[:, cc, :],
                            start=(cc == 0),
                            stop=(cc == CC - 1),
                        )
                    o_sc = outst.tile([P, 512], F32, name="o_sc")
                    nc.vector.scalar_tensor_tensor(
                        out=o_sc[:],
                        in0=psw[:],
                        scalar=1.0,
                        in1=rcB[:],
                        op0=Alu.mult,
                        op1=Alu.mult,
                    )
                    o_out = outst.tile([P, 512], F32)
                    nc.vector.scalar_tensor_tensor(
                        out=o_out[:],
                        in0=o_sc[:],
                        scalar=boCol[:, oc : oc + 1],
                        in1=xo_sb[:, oc, sl * 512 : (sl + 1) * 512],
                        op0=Alu.add,
                        op1=Alu.add,
                    )
                    nc.sync.dma_start(
                        out[oc * P : (oc + 1) * P, sl * 512 : (sl + 1) * 512], o_out[:]
                    )

            # ---------- slice 0 (query cols 0..511, frames 0..3) ----------
            # B1: scores+exp+rowsum only (buffered) so the PE keeps busy while
            # AG_V is still in flight; B2: the deferred attn@V matmuls.
            psum_o0 = [
                ppO.tile([P, 512], F32, tag=f"o{cc}", name=f"psum_o0{cc}")
                for cc in range(CC)
            ]
            psum_sum0 = ppSum.tile([1, 512], F32, tag="sum", name="psum_sum0")
            n0 = 0
            for f in range(4):
                qlo = P * f
                qw = 512 - qlo
                for t in range(NT):
                    pss = ppSc.tile([P, 512], F32, tag="sc", name="pss")
                    for cc in range(CC):
                        nc.tensor.matmul(
                            pss[:, :qw],
                            lhsT=K_sb[:, cc, f * NHW + t * P : f * NHW + (t + 1) * P],
                            rhs=Q_sb[:, cc, qlo:512],
                            start=(cc == 0),
                            stop=(cc == CC - 1),
                        )
                    nc.scalar.activation(
                        at0_sb[:, n0, :qw], pss[:, :qw], Act.Exp,
                        scale=float(INV_SQRT_C),
                    )
                    nc.tensor.matmul(
                        psum_sum0[:, qlo:],
                        lhsT=ones_bf[:],
                        rhs=at0_sb[:, n0, :qw],
                        start=(n0 == 0),
                        stop=(n0 == 4 * NT - 1),
                    )
                    n0 += 1
            rc0 = rows2.tile([1, 512], F32)
            nc.vector.reciprocal_approx_fast(out=rc0[:], in_=psum_sum0[:])
            nc.sync.dma_start(rc_dram[:, 0:512], rc0[:])
            rcB0 = rcb.tile([P, 512], F32)
            nc.sync.dma_start(rcB0[:], _bcast(rc_dram[0, 0:512]))
            n0 = 0
            for f in range(4):
                qlo = P * f
                qw = 512 - qlo
                for t in range(NT):
                    vt_t = vload.tile([P, C], KV_DT)
                    nc.sync.dma_start(vt_t[:], agv_out[f, t * P : (t + 1) * P, :])
                    for cc in range(CC):
                        nc.tensor.matmul(
                            psum_o0[cc][:, qlo:],
                            lhsT=vt_t[:, cc * P : (cc + 1) * P],
                            rhs=at0_sb[:, n0, :qw],
                            start=(n0 == 0),
                            stop=(n0 == 4 * NT - 1),
                        )
                    n0 += 1
            onrm0 = onorm.tile([P, CC, 512], BF16)
            for cc in range(CC):
                nc.scalar.activation(onrm0[:, cc, :], psum_o0[cc][:], Act.Copy)
            _tail(0, rcB0, onrm0)

            # ---------- slice 1 (query cols 512..1023, frames 0..7) ----------
            psum_o1 = [
                ppO.tile([P, 512], F32, tag=f"o{cc}", name=f"psum_o1{cc}")
                for cc in range(CC)
            ]
            psum_sum1 = ppSum.tile([1, 512], F32, tag="sum", name="psum_sum1")
            for f in range(NF):
                qlo = max(P * f - 512, 0)
                qw = 512 - qlo
                for t in range(NT):
                    vt_t = vload.tile([P, C], KV_DT)
                    nc.sync.dma_start(vt_t[:], agv_out[f, t * P : (t + 1) * P, :])

                    pss = ppSc.tile([P, 512], F32, tag="sc", name="pss")
                    for cc in range(CC):
                        nc.tensor.matmul(
                            pss[:, :qw],
                            lhsT=K_sb[:, cc, f * NHW + t * P : f * NHW + (t + 1) * P],
                            rhs=Q_sb[:, cc, 512 + qlo : 1024],
                            start=(cc == 0),
                            stop=(cc == CC - 1),
                        )
                    at = atp.tile([P, 512], BF16)
                    nc.scalar.activation(
                        at[:, :qw], pss[:, :qw], Act.Exp, scale=float(INV_SQRT_C)
                    )
                    first = f == 0 and t == 0
                    last = f == NF - 1 and t == NT - 1
                    nc.tensor.matmul(
                        psum_sum1[:, qlo:],
                        lhsT=ones_bf[:],
                        rhs=at[:, :qw],
                        start=first,
                        stop=last,
                    )
                    for cc in range(CC):
                        nc.tensor.matmul(
                            psum_o1[cc][:, qlo:],
                            lhsT=vt_t[:, cc * P : (cc + 1) * P],
                            rhs=at[:, :qw],
                            start=first,
                            stop=last,
                        )
            rc1 = rows2.tile([1, 512], F32)
            nc.vector.reciprocal_approx_fast(out=rc1[:], in_=psum_sum1[:])
            nc.sync.dma_start(rc_dram[:, 512:1024], rc1[:])
            rcB1 = rcb.tile([P, 512], F32)
            nc.sync.dma_start(rcB1[:], _bcast(rc_dram[0, 512:1024]))
            onrm1 = onorm.tile([P, CC, 512], BF16)
            for cc in range(CC):
                nc.scalar.activation(onrm1[:, cc, :], psum_o1[cc][:], Act.Copy)
            _tail(1, rcB1, onrm1)


def kernel(x, gamma, wq, bq, wk, bk, wv, bv, wo, bo):
    global LAST_RESULTS
    _ensure_axon_hooks()
    x = np.asarray(x, dtype=np.float32)
    gamma = np.asarray(gamma, dtype=np.float32).reshape(C)
    ws = {
        "wqT": np.ascontiguousarray(np.asarray(wq, np.float32).T).astype(ml_dtypes.bfloat16),
        "wkT": np.ascontiguousarray(np.asarray(wk, np.float32).T).astype(ml_dtypes.bfloat16),
        "wvT": np.ascontiguousarray(np.asarray(wv, np.float32).T).astype(ml_dtypes.bfloat16),
        "woT": np.ascontiguousarray(np.asarray(wo, np.float32).T).astype(ml_dtypes.bfloat16),
    }
    bs = {
        "bq": np.asarray(bq, np.float32).reshape(C),
        "bk": np.asarray(bk, np.float32).reshape(C),
        "bv": np.asarray(bv, np.float32).reshape(C),
        "bo": np.asarray(bo, np.float32).reshape(C),
    }

    xs = x.reshape(C, SEQ)  # [c, f*h*w], frame = s // 1024
    xsub = xs.reshape(C, NF, NHW // P, P)  # [c, frame, subchunk, 128]

    in_maps = []
    for i in range(NCORES):
        xo_i = np.ascontiguousarray(xsub[:, :, i, :]).reshape(C, NHW)
        xfi_i = np.ascontiguousarray(xs[:, i * NHW : (i + 1) * NHW]).astype(
            ml_dtypes.bfloat16
        )
        in_maps.append(
            {"xfi": xfi_i, "xo": xo_i, "gamma": gamma, **ws, **bs}
        )

    nc = _build_nc()
    nc.finalize()  # run Bacc passes (multi-wait splitting etc.) before lowering
    res = run_bass_kernel_spmd(nc, in_maps, list(range(NCORES)))
    LAST_RESULTS = res

    out_full = np.empty((C, SEQ), np.float32)
    ov = out_full.reshape(C, NF, NHW // P, P)
    for i in range(NCORES):
        ov[:, :, i, :] = res.results[i]["out"].reshape(C, NF, P)
    return out_full.reshape(B, C, NF, H, W)


# revision 15
# speedup vs baseline: 1.3695x; 1.3695x over previous
"""Trainium2 Bass kernel for AttnBlock (rmsnorm -> qkv -> block-causal frame
attention -> output proj -> residual).

Sharding (v4, sequence-parallel per the hint):
  * Queries: core i owns the i-th 128-token slice of every frame (8 query
    chunks of 128).  Query chunk s (frame s) attends frames 0..s, so spans are
    compile-time constants, identical on every core -> clean SPMD.
  * K/V projections: core i computes K/V ONLY for frame i (1024 tokens), then
    two AllGathers (K first, then V) distribute them in fp8e4.  A tiny warmup
    AllGather at t=0 absorbs the ~30us first-collective ncfw latency.
  * Everything on the PE runs fp8e4 with perf_mode=DoubleRow (256-wide
    contraction, 2x throughput).  Scores here are tiny (sigma~0.2, softmax
    nearly flat over thousands of keys), so fp8 quantization of q/k/v/at
    averages out to ~1e-3 relative output error vs the 2e-2 gate.
  * Phase B is split: B1 computes ALL exp(scores) (both query slices) into an
    SBUF buffer + row-sums while AG_V is still in flight; B2 then runs all
    attn@V contractions + output projection.

Accumulation is fp32 in PSUM.  Softmax skips max-subtraction; row sums via
ones-vector matmul on PE; 1/sum is applied after attn@V (commutes with WO);
row broadcasts (rms scale, 1/sum) are ones-column matmuls into PSUM instead of
DRAM round-trips.
"""

import math
import os
import sys

import numpy as np

for _p in ("/opt/trn_rl_repo",):
    if _p not in sys.path:
        sys.path.insert(0, _p)

import ml_dtypes  # noqa: E402

import concourse.bass as bass  # noqa: E402
import concourse.tile as tile  # noqa: E402
from concourse import bacc  # noqa: E402
from concourse import mybir  # noqa: E402
from concourse.bass_utils import run_bass_kernel_spmd  # noqa: E402

BF16 = mybir.dt.bfloat16
FP8 = mybir.dt.float8e4
F32 = mybir.dt.float32
DR = mybir.MatmulPerfMode.DoubleRow

B, C, NF, H, W = 1, 512, 8, 32, 32
NHW = H * W          # 1024 tokens per frame
SEQ = NF * NHW       # 8192
NCORES = 8
P = 128              # partitions
CC = C // P          # 4 channel chunks
NT = NHW // P        # 8 key tiles per frame
NP2 = NT // 2        # 4 key PAIR-tiles (256 keys) per frame
SQRT_C = math.sqrt(C)
INV_SQRT_C = 1.0 / SQRT_C

LAST_RESULTS = None  # BassKernelResults of the most recent run (for test.py)


def _ensure_axon_hooks():
    """bass_utils' trace path imports antenv.axon_hooks, which is absent from
    some container snapshots.  Provide the tiny registry (and wire the ctypes
    NTFF hook from trn_agent_boot when available) so tracing degrades
    gracefully instead of crashing."""
    import types

    try:
        import antenv.axon_hooks  # noqa: F401

        return
    except Exception:
        pass
    try:
        import antenv
    except Exception:
        antenv = types.ModuleType("antenv")
        sys.modules["antenv"] = antenv
    mod = types.ModuleType("antenv.axon_hooks")
    _h = [None]
    mod.set_axon_ntff_profile_hook = lambda hook: _h.__setitem__(0, hook)
    mod.get_axon_ntff_profile_hook = lambda: _h[0]
    sys.modules["antenv.axon_hooks"] = mod
    antenv.axon_hooks = mod
    try:
        from trn_agent_boot.trn_boot import _ntff_profile_via_ctypes

        hook = _ntff_profile_via_ctypes("/opt/axon/libaxon_pjrt.so")
        if hook is not None:
            mod.set_axon_ntff_profile_hook(hook)
    except Exception:
        pass


def _bcast(ap, p=P):
    """AP that reads a DRAM row and replicates it across p partitions."""
    return bass.AP(tensor=ap.tensor, offset=ap.offset, ap=[[0, p], *list(ap.ap)])


def _build_nc():
    nc = bacc.Bacc(num_devices=NCORES)

    xfi = nc.declare_dram_parameter("xfi", [C, NHW], BF16, isOutput=False)
    xo = nc.declare_dram_parameter("xo", [C, NHW], F32, isOutput=False)
    wqT = nc.declare_dram_parameter("wqT", [C, C], FP8, isOutput=False)
    wkT = nc.declare_dram_parameter("wkT", [C, C], FP8, isOutput=False)
    wvT = nc.declare_dram_parameter("wvT", [C, C], FP8, isOutput=False)
    woT = nc.declare_dram_parameter("woT", [C, C], FP8, isOutput=False)
    gamma = nc.declare_dram_parameter("gamma", [C], F32, isOutput=False)
    bq = nc.declare_dram_parameter("bq", [C], F32, isOutput=False)
    bk = nc.declare_dram_parameter("bk", [C], F32, isOutput=False)
    bv = nc.declare_dram_parameter("bv", [C], F32, isOutput=False)
    bo = nc.declare_dram_parameter("bo", [C], F32, isOutput=False)
    out = nc.declare_dram_parameter("out", [C, NHW], F32, isOutput=True)

    with tile.TileContext(nc) as tc:
        _emit(tc, xfi, xo, wqT, wkT, wvT, woT, gamma, bq, bk, bv, bo, out)
    return nc


def _frames_qw(sl):
    """(frame, qlo, qw) list for a 512-query slice."""
    out = []
    for f in range(NF if sl else 4):
        qlo = max(P * f - 512 * sl, 0)
        out.append((f, qlo, 512 - qlo))
    return out


def _emit(tc, xfi, xo, wqT, wkT, wvT, woT, gamma, bq, bk, bv, bo, out):
    nc = tc.nc
    Act = mybir.ActivationFunctionType
    Alu = mybir.AluOpType
    RG = [[i for i in range(NCORES)]]

    with (
        tc.tile_pool(name="dram", bufs=1, space="DRAM") as drp,
        tc.tile_pool(name="singles", bufs=1) as singles,
    ):
        # ---- DRAM scratch ----
        agw_in = drp.tile([1, 128], F32)
        agw_out = drp.tile([NCORES, 128], F32, addr_space="Shared")
        agk_in = drp.tile([C, NHW], FP8)
        agk_out = drp.tile([NCORES, C, NHW], FP8, addr_space="Shared")
        agv_in = drp.tile([NHW, C], FP8)
        agv_out = drp.tile([NCORES, NHW, C], FP8, addr_space="Shared")

        # ---- warmup collective (absorbs ncfw first-call latency) ----
        warm_sb = singles.tile([1, 128], F32)
        nc.vector.memset(warm_sb[:], 0.0)
        nc.gpsimd.dma_start(agw_in[:], warm_sb[:])
        nc.gpsimd.collective_compute(
            "AllGather",
            mybir.AluOpType.bypass,
            replica_groups=RG,
            ins=[agw_in[:].opt()],
            outs=[agw_out[:].opt()],
        )

        # ---- constants & weights in SBUF ----
        def _wsb(wT, eng=None):
            t = singles.tile([P, CC, C], FP8, tag=f"w_{wT.name}")
            (eng or nc.sync).dma_start(t[:], wT[:].rearrange("(cc p) o -> p cc o", p=P))
            return t

        def _col(v):
            t = singles.tile([P, CC], F32, tag=f"col_{v.name}")
            nc.scalar.dma_start(t[:], v[:].rearrange("(cc p) -> p cc", p=P))
            return t

        bqCol, bkCol, boCol = _col(bq), _col(bk), _col(bo)

        bvB = singles.tile([P, C], F32)             # bv broadcast along partitions
        nc.scalar.dma_start(bvB[:], _bcast(bv[:]))

        ones2 = singles.tile([P, 2, 16], FP8)       # DoubleRow ones column
        nc.vector.memset(ones2[:], 1.0)
        ones_row = singles.tile([1, P], F32)        # broadcast-matmul row
        nc.vector.memset(ones_row[:], 1.0)
        eps_row = singles.tile([1, 1], F32)
        nc.vector.memset(eps_row[:], 1e-24)

        # ---- big persistent tensors ----
        K_sb = singles.tile([P, CC, SEQ], FP8)      # gathered keys [c_chunk, seq]
        Q_sb = singles.tile([P, CC, NHW], FP8)      # own queries
        xo_sb = singles.tile([P, CC, NHW], F32)     # own x (residual)
        at_sb = singles.tile([P, 48, 2, 512], FP8)  # all exp(scores) pair-tiles

        # ================= phase A: norm + projections + gathers ============
        with (
            tc.tile_pool(name="stage", bufs=2) as stage,
            tc.tile_pool(name="hpool", bufs=2) as hpool,
            tc.tile_pool(name="rows", bufs=3) as rows,
            tc.tile_pool(name="vstage", bufs=4) as vstage,
            tc.tile_pool(name="ppA", bufs=3, space="PSUM") as ppA,
            tc.tile_pool(name="ppS", bufs=2, space="PSUM") as ppS,
            tc.tile_pool(name="ppR", bufs=2, space="PSUM") as ppR,
        ):
            # DMA priority order: xfi is the critical path to AG_K.
            xfi_sb = stage.tile([P, CC, NHW], BF16, tag="xfi")
            nc.sync.dma_start(xfi_sb[:], xfi[:].rearrange("(cc p) s -> p cc s", p=P))
            wk_sb, wv_sb = _wsb(wkT), _wsb(wvT)

            def _norm_half(x_in, pc):
                """rms rows for a 512-token half -> [P, 512] broadcast in PSUM."""
                x2 = stage.tile([P, CC, 512], FP8, tag="x2")
                nc.scalar.activation(x2[:], x_in[:], Act.Square)
                ps = ppS.tile([1, 512], F32)
                for g in range(2):
                    nc.tensor.matmul(
                        ps[:],
                        lhsT=ones2[:, :, 0:1],
                        rhs=x2[:, 2 * g : 2 * g + 2, :],
                        start=(g == 0),
                        stop=(g == 1),
                        perf_mode=DR,
                    )
                nrm = rows.tile([1, 512], F32, tag="nrm")
                nc.scalar.activation(nrm[:], ps[:], Act.Sqrt, bias=eps_row[:])
                rrow = rows.tile([1, 512], F32, tag="rrow")
                nc.vector.reciprocal_approx_fast(out=rrow[:], in_=nrm[:])
                rB = ppR.tile([P, 512], F32)
                nc.tensor.matmul(rB[:], lhsT=ones_row[:], rhs=rrow[:])
                return rB

            def _h_from(xsrc):
                ht = hpool.tile([P, CC, NHW], FP8)
                for pc in range(2):
                    rB = _norm_half(xsrc[:, :, pc * 512 : (pc + 1) * 512], pc)
                    for cc in range(CC):
                        nc.vector.scalar_tensor_tensor(
                            out=ht[:, cc, pc * 512 : (pc + 1) * 512],
                            in0=xsrc[:, cc, pc * 512 : (pc + 1) * 512],
                            scalar=float(SQRT_C),
                            in1=rB[:],
                            op0=Alu.mult,
                            op1=Alu.mult,
                        )
                return ht

            h_i = _h_from(xfi_sb)

            # ---- K_i projection -> agk_in (half-granularity) -> AllGather K
            ksb_i = stage.tile([P, CC, NHW], FP8, tag="ksb")
            agk_in_r = agk_in[:].rearrange("(cc p) s -> p cc s", p=P)
            for pc in range(2):
                for oc in range(CC):
                    psk = ppA.tile([P, 512], F32, tag="ps", name="psk")
                    for g in range(2):
                        nc.tensor.matmul(
                            psk[:],
                            lhsT=wk_sb[:, 2 * g : 2 * g + 2, oc * P : (oc + 1) * P],
                            rhs=h_i[:, 2 * g : 2 * g + 2, pc * 512 : (pc + 1) * 512],
                            start=(g == 0),
                            stop=(g == 1),
                            perf_mode=DR,
                        )
                    nc.scalar.activation(
                        ksb_i[:, oc, pc * 512 : (pc + 1) * 512],
                        psk[:],
                        Act.Identity,
                        bias=bkCol[:, oc : oc + 1],
                        scale=1.0 / 16.0,
                    )
                nc.sync.dma_start(
                    agk_in_r[:, :, pc * 512 : (pc + 1) * 512],
                    ksb_i[:, :, pc * 512 : (pc + 1) * 512],
                )
            nc.gpsimd.collective_compute(
                "AllGather",
                mybir.AluOpType.bypass,
                replica_groups=RG,
                ins=[agk_in[:].opt()],
                outs=[agk_out[:].opt()],
            )

            # ---- V_i projection -> agv_in -> AllGather V ----
            for t in range(NT):
                psv = ppA.tile([P, 512], F32, tag="ps", name="psv")
                for g in range(2):
                    nc.tensor.matmul(
                        psv[:],
                        lhsT=h_i[:, 2 * g : 2 * g + 2, t * P : (t + 1) * P],
                        rhs=wv_sb[:, 2 * g : 2 * g + 2, :],
                        start=(g == 0),
                        stop=(g == 1),
                        perf_mode=DR,
                    )
                vt_s = vstage.tile([P, C], FP8)
                nc.vector.scalar_tensor_tensor(
                    out=vt_s[:],
                    in0=psv[:],
                    scalar=1.0 / 16.0,
                    in1=bvB[:],
                    op0=Alu.mult,
                    op1=Alu.add,
                )
                nc.sync.dma_start(agv_in[t * P : (t + 1) * P, :], vt_s[:])
            nc.gpsimd.collective_compute(
                "AllGather",
                mybir.AluOpType.bypass,
                replica_groups=RG,
                ins=[agv_in[:].opt()],
                outs=[agv_out[:].opt()],
            )

            # ---- own norm + h + Q projection (fills the gather window) ----
            nc.scalar.dma_start(xo_sb[:], xo[:].rearrange("(cc p) s -> p cc s", p=P))
            wq_sb, wo_sb = _wsb(wqT, nc.scalar), _wsb(woT, nc.scalar)
            xbf = stage.tile([P, CC, NHW], BF16, tag="xbf")
            nc.vector.tensor_copy(out=xbf[:], in_=xo_sb[:])
            h_own = _h_from(xbf)
            for oc in range(CC):
                for pc in range(2):
                    psq = ppA.tile([P, 512], F32, tag="ps", name="psq")
                    for g in range(2):
                        nc.tensor.matmul(
                            psq[:],
                            lhsT=wq_sb[:, 2 * g : 2 * g + 2, oc * P : (oc + 1) * P],
                            rhs=h_own[:, 2 * g : 2 * g + 2, pc * 512 : (pc + 1) * 512],
                            start=(g == 0),
                            stop=(g == 1),
                            perf_mode=DR,
                        )
                    nc.scalar.activation(
                        Q_sb[:, oc, pc * 512 : (pc + 1) * 512],
                        psq[:],
                        Act.Identity,
                        bias=bqCol[:, oc : oc + 1],
                        scale=1.0 / 16.0,
                    )

            # gathered K -> SBUF, one DMA per frame (pipelines with attention;
            # emitted last so no phase-A DMA queues behind the AG_K wait)
            for j in range(NF):
                nc.sync.dma_start(
                    K_sb[:, :, j * NHW : (j + 1) * NHW],
                    agk_out[j].rearrange("(cc p) s -> p cc s", p=P),
                )

        if os.environ.get("BASS_PHASE") == "A":
            with tc.tile_pool(name="dummy", bufs=1) as dummy:
                dmy = dummy.tile([P, CC, NHW], F32)
                nc.vector.tensor_copy(out=dmy[:], in_=xo_sb[:])
                nc.sync.dma_start(out[:].rearrange("(cc p) s -> p cc s", p=P), dmy[:])
            return

        # ========== phase B1: all scores + exp + row-sums (both slices) =====
        rc = [
            singles.tile([1, 512], F32, tag=f"rc{sl}", name=f"rc{sl}")
            for sl in range(2)
        ]
        with (
            tc.tile_pool(name="ppSc", bufs=3, space="PSUM") as ppSc,
            tc.tile_pool(name="ppSum", bufs=2, space="PSUM") as ppSum,
        ):
            for sl in range(2):
                fq = _frames_qw(sl)
                psum_sum = ppSum.tile([1, 512], F32, tag="sum", name=f"psum_sum{sl}")
                npair = len(fq) * NP2
                n = 0
                for f, qlo, qw in fq:
                    for t2 in range(NP2):
                        pss2 = ppSc.tile([P, 2, 512], F32, tag="sc", name="pss2")
                        for j in range(2):
                            k0 = f * NHW + (t2 * 2 + j) * P
                            for g in range(2):
                                nc.tensor.matmul(
                                    pss2[:, j, :qw],
                                    lhsT=K_sb[:, 2 * g : 2 * g + 2, k0 : k0 + P],
                                    rhs=Q_sb[:, 2 * g : 2 * g + 2,
                                             sl * 512 + qlo : (sl + 1) * 512],
                                    start=(g == 0),
                                    stop=(g == 1),
                                    perf_mode=DR,
                                )
                        idx = sl * 16 + f * NP2 + t2
                        nc.scalar.activation(
                            at_sb[:, idx, :, :qw], pss2[:, :, :qw], Act.Exp,
                            scale=float(INV_SQRT_C),
                        )
                        nc.tensor.matmul(
                            psum_sum[:, qlo:],
                            lhsT=ones2[:, :, 0:1],
                            rhs=at_sb[:, idx, :, :qw],
                            start=(n == 0),
                            stop=(n == npair - 1),
                            perf_mode=DR,
                        )
                        n += 1
                nc.vector.reciprocal_approx_fast(out=rc[sl][:], in_=psum_sum[:])

        # ========== phase B2: attn@V + output projection ====================
        with (
            tc.tile_pool(name="vload", bufs=6) as vload,
            tc.tile_pool(name="onorm", bufs=2) as onorm,
            tc.tile_pool(name="outst", bufs=2) as outst,
            tc.tile_pool(name="ppO", bufs=1, space="PSUM") as ppO,
            tc.tile_pool(name="ppW", bufs=2, space="PSUM") as ppW,
            tc.tile_pool(name="ppR2", bufs=2, space="PSUM") as ppR2,
        ):
            for sl in range(2):
                fq = _frames_qw(sl)
                psum_o = [
                    ppO.tile([P, 512], F32, tag=f"o{cc}", name=f"psum_o{sl}{cc}")
                    for cc in range(CC)
                ]
                rcB_ps = ppR2.tile([P, 512], F32, tag="rcb", name=f"rcB_ps{sl}")
                nc.tensor.matmul(rcB_ps[:], lhsT=ones_row[:], rhs=rc[sl][:])
                rcB = onorm.tile([P, 512], F32, tag="rcbs", name=f"rcB{sl}")
                nc.vector.tensor_copy(out=rcB[:], in_=rcB_ps[:])
                npair = len(fq) * NP2
                n = 0
                for f, qlo, qw in fq:
                    for t2 in range(NP2):
                        vt2 = vload.tile([P, 2, C], FP8)
                        nc.sync.dma_start(
                            vt2[:],
                            agv_out[f, t2 * 256 : (t2 + 1) * 256, :].rearrange(
                                "(j p) c -> p j c", p=P
                            ),
                        )
                        idx = sl * 16 + f * NP2 + t2
                        for cc in range(CC):
                            nc.tensor.matmul(
                                psum_o[cc][:, qlo:],
                                lhsT=vt2[:, :, cc * P : (cc + 1) * P],
                                rhs=at_sb[:, idx, :, :qw],
                                start=(n == 0),
                                stop=(n == npair - 1),
                                perf_mode=DR,
                            )
                        n += 1
                onrm = onorm.tile([P, CC, 512], FP8)
                for cc in range(CC):
                    nc.scalar.activation(
                        onrm[:, cc, :], psum_o[cc][:], Act.Identity,
                        scale=1.0 / 16.0,
                    )
                # output projection + 1/sum + bias + residual
                for oc in range(CC):
                    psw = ppW.tile([P, 512], F32, tag="sc", name="psw")
                    for g in range(2):
                        nc.tensor.matmul(
                            psw[:],
                            lhsT=wo_sb[:, 2 * g : 2 * g + 2, oc * P : (oc + 1) * P],
                            rhs=onrm[:, 2 * g : 2 * g + 2, :],
                            start=(g == 0),
                            stop=(g == 1),
                            perf_mode=DR,
                        )
                    o_sc = outst.tile([P, 512], F32, name="o_sc")
                    nc.vector.scalar_tensor_tensor(
                        out=o_sc[:],
                        in0=psw[:],
                        scalar=1.0,
                        in1=rcB[:],
                        op0=Alu.mult,
                        op1=Alu.mult,
                    )
                    o_out = outst.tile([P, 512], F32)
                    nc.vector.scalar_tensor_tensor(
                        out=o_out[:],
                        in0=o_sc[:],
                        scalar=boCol[:, oc : oc + 1],
                        in1=xo_sb[:, oc, sl * 512 : (sl + 1) * 512],
                        op0=Alu.add,
                        op1=Alu.add,
                    )
                    nc.sync.dma_start(
                        out[oc * P : (oc + 1) * P, sl * 512 : (sl + 1) * 512], o_out[:]
                    )


def kernel(x, gamma, wq, bq, wk, bk, wv, bv, wo, bo):
    global LAST_RESULTS
    _ensure_axon_hooks()
    x = np.asarray(x, dtype=np.float32)
    gamma = np.asarray(gamma, dtype=np.float32).reshape(C)
    f8 = ml_dtypes.float8_e4m3
    # x16 scale keeps the sigma=0.02 weights out of fp8's subnormal range
    # (unscaled on-device at each PSUM->SBUF copy); gamma is folded into the
    # input-channel rows of the q/k/v weights (h = normalize(x)*sqrt(C) only).
    g = np.asarray(gamma, np.float32).reshape(C, 1)
    ws = {
        "wqT": np.ascontiguousarray(np.asarray(wq, np.float32).T * g * 16.0).astype(f8),
        "wkT": np.ascontiguousarray(np.asarray(wk, np.float32).T * g * 16.0).astype(f8),
        "wvT": np.ascontiguousarray(np.asarray(wv, np.float32).T * g * 16.0).astype(f8),
        "woT": np.ascontiguousarray(np.asarray(wo, np.float32).T * 16.0).astype(f8),
    }
    bs = {
        "bq": np.asarray(bq, np.float32).reshape(C),
        "bk": np.asarray(bk, np.float32).reshape(C),
        "bv": np.asarray(bv, np.float32).reshape(C),
        "bo": np.asarray(bo, np.float32).reshape(C),
    }

    xs = x.reshape(C, SEQ)  # [c, f*h*w], frame = s // 1024
    xsub = xs.reshape(C, NF, NHW // P, P)  # [c, frame, subchunk, 128]

    in_maps = []
    for i in range(NCORES):
        xo_i = np.ascontiguousarray(xsub[:, :, i, :]).reshape(C, NHW)
        xfi_i = np.ascontiguousarray(xs[:, i * NHW : (i + 1) * NHW]).astype(
            ml_dtypes.bfloat16
        )
        in_maps.append(
            {"xfi": xfi_i, "xo": xo_i, "gamma": gamma, **ws, **bs}
        )

    nc = _build_nc()
    nc.finalize()  # run Bacc passes (multi-wait splitting etc.) before lowering
    res = run_bass_kernel_spmd(nc, in_maps, list(range(NCORES)))
    LAST_RESULTS = res

    out_full = np.empty((C, SEQ), np.float32)
    ov = out_full.reshape(C, NF, NHW // P, P)
    for i in range(NCORES):
        ov[:, :, i, :] = res.results[i]["out"].reshape(C, NF, P)
    return out_full.reshape(B, C, NF, H, W)


# revision 16
# speedup vs baseline: 1.4695x; 1.0730x over previous
"""Trainium2 Bass kernel for AttnBlock (rmsnorm -> qkv -> block-causal frame
attention -> output proj -> residual).

Sharding (v4, sequence-parallel per the hint):
  * Queries: core i owns the i-th 128-token slice of every frame (8 query
    chunks of 128).  Query chunk s (frame s) attends frames 0..s, so spans are
    compile-time constants, identical on every core -> clean SPMD.
  * K/V projections: core i computes K/V ONLY for frame i (1024 tokens), then
    two AllGathers (K first, then V) distribute them in fp8e4.  A tiny warmup
    AllGather at t=0 absorbs the ~30us first-collective ncfw latency.
  * Everything on the PE runs fp8e4 with perf_mode=DoubleRow (256-wide
    contraction, 2x throughput).  Scores here are tiny (sigma~0.2, softmax
    nearly flat over thousands of keys), so fp8 quantization of q/k/v/at
    averages out to ~1e-3 relative output error vs the 2e-2 gate.
  * Phase B is split: B1 computes ALL exp(scores) (both query slices) into an
    SBUF buffer + row-sums while AG_V is still in flight; B2 then runs all
    attn@V contractions + output projection.

Accumulation is fp32 in PSUM.  Softmax skips max-subtraction; row sums via
ones-vector matmul on PE; 1/sum is applied after attn@V (commutes with WO);
row broadcasts (rms scale, 1/sum) are ones-column matmuls into PSUM instead of
DRAM round-trips.
"""

import math
import os
import sys

import numpy as np

for _p in ("/opt/trn_rl_repo",):
    if _p not in sys.path:
        sys.path.insert(0, _p)

import ml_dtypes  # noqa: E402

import concourse.bass as bass  # noqa: E402
import concourse.tile as tile  # noqa: E402
from concourse import bacc  # noqa: E402
from concourse import mybir  # noqa: E402
from concourse.bass_utils import run_bass_kernel_spmd  # noqa: E402

BF16 = mybir.dt.bfloat16
FP8 = mybir.dt.float8e4
F32 = mybir.dt.float32
DR = mybir.MatmulPerfMode.DoubleRow

B, C, NF, H, W = 1, 512, 8, 32, 32
NHW = H * W          # 1024 tokens per frame
SEQ = NF * NHW       # 8192
NCORES = 8
P = 128              # partitions
CC = C // P          # 4 channel chunks
NT = NHW // P        # 8 key tiles per frame
NP2 = NT // 2        # 4 key PAIR-tiles (256 keys) per frame
SQRT_C = math.sqrt(C)
INV_SQRT_C = 1.0 / SQRT_C

LAST_RESULTS = None  # BassKernelResults of the most recent run (for test.py)


def _ensure_axon_hooks():
    """bass_utils' trace path imports antenv.axon_hooks, which is absent from
    some container snapshots.  Provide the tiny registry (and wire the ctypes
    NTFF hook from trn_agent_boot when available) so tracing degrades
    gracefully instead of crashing."""
    import types

    try:
        import antenv.axon_hooks  # noqa: F401

        return
    except Exception:
        pass
    try:
        import antenv
    except Exception:
        antenv = types.ModuleType("antenv")
        sys.modules["antenv"] = antenv
    mod = types.ModuleType("antenv.axon_hooks")
    _h = [None]
    mod.set_axon_ntff_profile_hook = lambda hook: _h.__setitem__(0, hook)
    mod.get_axon_ntff_profile_hook = lambda: _h[0]
    sys.modules["antenv.axon_hooks"] = mod
    antenv.axon_hooks = mod
    try:
        from trn_agent_boot.trn_boot import _ntff_profile_via_ctypes

        hook = _ntff_profile_via_ctypes("/opt/axon/libaxon_pjrt.so")
        if hook is not None:
            mod.set_axon_ntff_profile_hook(hook)
    except Exception:
        pass


def _bcast(ap, p=P):
    """AP that reads a DRAM row and replicates it across p partitions."""
    return bass.AP(tensor=ap.tensor, offset=ap.offset, ap=[[0, p], *list(ap.ap)])


def _build_nc():
    nc = bacc.Bacc(num_devices=NCORES)

    xfi = nc.declare_dram_parameter("xfi", [C, NHW], BF16, isOutput=False)
    xo = nc.declare_dram_parameter("xo", [C, NHW], F32, isOutput=False)
    wqT = nc.declare_dram_parameter("wqT", [C, C], FP8, isOutput=False)
    wkT = nc.declare_dram_parameter("wkT", [C, C], FP8, isOutput=False)
    wvT = nc.declare_dram_parameter("wvT", [C, C], FP8, isOutput=False)
    woT = nc.declare_dram_parameter("woT", [C, C], FP8, isOutput=False)
    gamma = nc.declare_dram_parameter("gamma", [C], F32, isOutput=False)
    bq = nc.declare_dram_parameter("bq", [C], F32, isOutput=False)
    bk = nc.declare_dram_parameter("bk", [C], F32, isOutput=False)
    bv = nc.declare_dram_parameter("bv", [C], F32, isOutput=False)
    bo = nc.declare_dram_parameter("bo", [C], F32, isOutput=False)
    out = nc.declare_dram_parameter("out", [C, NHW], F32, isOutput=True)

    with tile.TileContext(nc) as tc:
        _emit(tc, xfi, xo, wqT, wkT, wvT, woT, gamma, bq, bk, bv, bo, out)
    return nc


def _frames_qw(sl):
    """(frame, qlo, qw) list for a 512-query slice."""
    out = []
    for f in range(NF if sl else 4):
        qlo = max(P * f - 512 * sl, 0)
        out.append((f, qlo, 512 - qlo))
    return out


def _emit(tc, xfi, xo, wqT, wkT, wvT, woT, gamma, bq, bk, bv, bo, out):
    nc = tc.nc
    Act = mybir.ActivationFunctionType
    Alu = mybir.AluOpType
    RG = [[i for i in range(NCORES)]]

    with (
        tc.tile_pool(name="dram", bufs=1, space="DRAM") as drp,
        tc.tile_pool(name="singles", bufs=1) as singles,
    ):
        # ---- DRAM scratch ----
        agk_in = drp.tile([2, C, 512], FP8)       # K halves (tokens 0-511, 512-1023)
        agk_out = [
            drp.tile([NCORES, C, 512], FP8, addr_space="Shared", name=f"agk_out{pc}")
            for pc in range(2)
        ]
        agv_in = drp.tile([NHW, C], FP8)
        agv_out = drp.tile([NCORES, NHW, C], FP8, addr_space="Shared")

        # ---- constants & weights in SBUF ----
        def _wsb(wT, eng=None):
            t = singles.tile([P, CC, C], FP8, tag=f"w_{wT.name}")
            (eng or nc.sync).dma_start(t[:], wT[:].rearrange("(cc p) o -> p cc o", p=P))
            return t

        def _col(v):
            t = singles.tile([P, CC], F32, tag=f"col_{v.name}")
            nc.scalar.dma_start(t[:], v[:].rearrange("(cc p) -> p cc", p=P))
            return t

        bqCol, bkCol, boCol = _col(bq), _col(bk), _col(bo)

        bvB = singles.tile([P, C], F32)             # bv broadcast along partitions
        nc.scalar.dma_start(bvB[:], _bcast(bv[:]))

        ones2 = singles.tile([P, 2, 16], FP8)       # DoubleRow ones column
        nc.vector.memset(ones2[:], 1.0)
        ones_row = singles.tile([1, P], F32)        # broadcast-matmul row
        nc.vector.memset(ones_row[:], 1.0)
        eps_row = singles.tile([1, 1], F32)
        nc.vector.memset(eps_row[:], 1e-24)

        # ---- big persistent tensors ----
        K_sb = singles.tile([P, CC, SEQ], FP8)      # gathered keys [c_chunk, seq]
        Q_sb = singles.tile([P, CC, NHW], FP8)      # own queries
        xo_sb = singles.tile([P, CC, NHW], F32)     # own x (residual)
        at_sb = singles.tile([P, 48, 2, 512], FP8)  # all exp(scores) pair-tiles

        # ================= phase A: norm + projections + gathers ============
        with (
            tc.tile_pool(name="stage", bufs=2) as stage,
            tc.tile_pool(name="hpool", bufs=2) as hpool,
            tc.tile_pool(name="rows", bufs=3) as rows,
            tc.tile_pool(name="vstage", bufs=4) as vstage,
            tc.tile_pool(name="ppA", bufs=3, space="PSUM") as ppA,
            tc.tile_pool(name="ppS", bufs=2, space="PSUM") as ppS,
            tc.tile_pool(name="ppR", bufs=2, space="PSUM") as ppR,
        ):
            # DMA priority order: xfi is the critical path to AG_K.
            xfi_sb = stage.tile([P, CC, NHW], BF16, tag="xfi")
            nc.sync.dma_start(xfi_sb[:], xfi[:].rearrange("(cc p) s -> p cc s", p=P))
            wk_sb, wv_sb = _wsb(wkT), _wsb(wvT)

            def _norm_half(x_in, pc):
                """rms rows for a 512-token half -> [P, 512] broadcast in PSUM."""
                x2 = stage.tile([P, CC, 512], FP8, tag="x2")
                nc.scalar.activation(x2[:], x_in[:], Act.Square)
                ps = ppS.tile([1, 512], F32)
                for g in range(2):
                    nc.tensor.matmul(
                        ps[:],
                        lhsT=ones2[:, :, 0:1],
                        rhs=x2[:, 2 * g : 2 * g + 2, :],
                        start=(g == 0),
                        stop=(g == 1),
                        perf_mode=DR,
                    )
                nrm = rows.tile([1, 512], F32, tag="nrm")
                nc.scalar.activation(nrm[:], ps[:], Act.Sqrt, bias=eps_row[:])
                rrow = rows.tile([1, 512], F32, tag="rrow")
                nc.vector.reciprocal_approx_fast(out=rrow[:], in_=nrm[:])
                rB = ppR.tile([P, 512], F32)
                nc.tensor.matmul(rB[:], lhsT=ones_row[:], rhs=rrow[:])
                return rB

            def _h_from(xsrc):
                ht = hpool.tile([P, CC, NHW], FP8)
                for pc in range(2):
                    rB = _norm_half(xsrc[:, :, pc * 512 : (pc + 1) * 512], pc)
                    for cc in range(CC):
                        nc.vector.scalar_tensor_tensor(
                            out=ht[:, cc, pc * 512 : (pc + 1) * 512],
                            in0=xsrc[:, cc, pc * 512 : (pc + 1) * 512],
                            scalar=float(SQRT_C),
                            in1=rB[:],
                            op0=Alu.mult,
                            op1=Alu.mult,
                        )
                return ht

            h_i = _h_from(xfi_sb)

            # ---- K_i projection -> agk_in (half-granularity) -> AllGather K
            ksb_i = stage.tile([P, CC, NHW], FP8, tag="ksb")
            for pc in range(2):
                for oc in range(CC):
                    psk = ppA.tile([P, 512], F32, tag="ps", name="psk")
                    for g in range(2):
                        nc.tensor.matmul(
                            psk[:],
                            lhsT=wk_sb[:, 2 * g : 2 * g + 2, oc * P : (oc + 1) * P],
                            rhs=h_i[:, 2 * g : 2 * g + 2, pc * 512 : (pc + 1) * 512],
                            start=(g == 0),
                            stop=(g == 1),
                            perf_mode=DR,
                        )
                    nc.scalar.activation(
                        ksb_i[:, oc, pc * 512 : (pc + 1) * 512],
                        psk[:],
                        Act.Identity,
                        bias=bkCol[:, oc : oc + 1],
                        scale=1.0 / 16.0,
                    )
                nc.sync.dma_start(
                    agk_in[pc].rearrange("(cc p) s -> p cc s", p=P),
                    ksb_i[:, :, pc * 512 : (pc + 1) * 512],
                )
                nc.gpsimd.collective_compute(
                    "AllGather",
                    mybir.AluOpType.bypass,
                    replica_groups=RG,
                    ins=[agk_in[pc][:].opt()],
                    outs=[agk_out[pc][:].opt()],
                )

            # ---- V_i projection -> agv_in -> AllGather V ----
            for t in range(NT):
                psv = ppA.tile([P, 512], F32, tag="ps", name="psv")
                for g in range(2):
                    nc.tensor.matmul(
                        psv[:],
                        lhsT=h_i[:, 2 * g : 2 * g + 2, t * P : (t + 1) * P],
                        rhs=wv_sb[:, 2 * g : 2 * g + 2, :],
                        start=(g == 0),
                        stop=(g == 1),
                        perf_mode=DR,
                    )
                vt_s = vstage.tile([P, C], FP8)
                nc.vector.scalar_tensor_tensor(
                    out=vt_s[:],
                    in0=psv[:],
                    scalar=1.0 / 16.0,
                    in1=bvB[:],
                    op0=Alu.mult,
                    op1=Alu.add,
                )
                nc.sync.dma_start(agv_in[t * P : (t + 1) * P, :], vt_s[:])
            nc.gpsimd.collective_compute(
                "AllGather",
                mybir.AluOpType.bypass,
                replica_groups=RG,
                ins=[agv_in[:].opt()],
                outs=[agv_out[:].opt()],
            )

            # ---- own norm + h + Q projection (fills the gather window) ----
            nc.scalar.dma_start(xo_sb[:], xo[:].rearrange("(cc p) s -> p cc s", p=P))
            wq_sb, wo_sb = _wsb(wqT, nc.scalar), _wsb(woT, nc.scalar)
            xbf = stage.tile([P, CC, NHW], BF16, tag="xbf")
            nc.vector.tensor_copy(out=xbf[:], in_=xo_sb[:])
            h_own = _h_from(xbf)
            for oc in range(CC):
                for pc in range(2):
                    psq = ppA.tile([P, 512], F32, tag="ps", name="psq")
                    for g in range(2):
                        nc.tensor.matmul(
                            psq[:],
                            lhsT=wq_sb[:, 2 * g : 2 * g + 2, oc * P : (oc + 1) * P],
                            rhs=h_own[:, 2 * g : 2 * g + 2, pc * 512 : (pc + 1) * 512],
                            start=(g == 0),
                            stop=(g == 1),
                            perf_mode=DR,
                        )
                    nc.scalar.activation(
                        Q_sb[:, oc, pc * 512 : (pc + 1) * 512],
                        psq[:],
                        Act.Identity,
                        bias=bqCol[:, oc : oc + 1],
                        scale=1.0 / 16.0,
                    )

            # gathered K -> SBUF, one DMA per frame (pipelines with attention;
            # emitted last so no phase-A DMA queues behind the AG_K wait)
            for pc in range(2):
                for j in range(NF):
                    nc.sync.dma_start(
                        K_sb[:, :, j * NHW + pc * 512 : j * NHW + (pc + 1) * 512],
                        agk_out[pc][j].rearrange("(cc p) s -> p cc s", p=P),
                    )

        if os.environ.get("BASS_PHASE") == "A":
            with tc.tile_pool(name="dummy", bufs=1) as dummy:
                dmy = dummy.tile([P, CC, NHW], F32)
                nc.vector.tensor_copy(out=dmy[:], in_=xo_sb[:])
                nc.sync.dma_start(out[:].rearrange("(cc p) s -> p cc s", p=P), dmy[:])
            return

        # ========== phase B1: all scores + exp + row-sums (both slices) =====
        rc = [
            singles.tile([1, 512], F32, tag=f"rc{sl}", name=f"rc{sl}")
            for sl in range(2)
        ]
        with (
            tc.tile_pool(name="ppSc", bufs=3, space="PSUM") as ppSc,
            tc.tile_pool(name="ppSum", bufs=2, space="PSUM") as ppSum,
        ):
            psum_sums = [
                ppSum.tile([1, 512], F32, tag="sum", name=f"psum_sum{sl}")
                for sl in range(2)
            ]
            ncnt = [0, 0]
            npair = [len(_frames_qw(sl)) * NP2 for sl in range(2)]
            for phase in range(2):
                for sl in range(2):
                    for f, qlo, qw in _frames_qw(sl):
                        for t2 in (2 * phase, 2 * phase + 1):
                            pss2 = ppSc.tile([P, 2, 512], F32, tag="sc", name="pss2")
                            for j in range(2):
                                k0 = f * NHW + (t2 * 2 + j) * P
                                for g in range(2):
                                    nc.tensor.matmul(
                                        pss2[:, j, :qw],
                                        lhsT=K_sb[:, 2 * g : 2 * g + 2, k0 : k0 + P],
                                        rhs=Q_sb[:, 2 * g : 2 * g + 2,
                                                 sl * 512 + qlo : (sl + 1) * 512],
                                        start=(g == 0),
                                        stop=(g == 1),
                                        perf_mode=DR,
                                    )
                            idx = sl * 16 + f * NP2 + t2
                            nc.scalar.activation(
                                at_sb[:, idx, :, :qw], pss2[:, :, :qw], Act.Exp,
                                scale=float(INV_SQRT_C),
                            )
                            nc.tensor.matmul(
                                psum_sums[sl][:, qlo:],
                                lhsT=ones2[:, :, 0:1],
                                rhs=at_sb[:, idx, :, :qw],
                                start=(ncnt[sl] == 0),
                                stop=(ncnt[sl] == npair[sl] - 1),
                                perf_mode=DR,
                            )
                            ncnt[sl] += 1
            for sl in range(2):
                nc.vector.reciprocal_approx_fast(out=rc[sl][:], in_=psum_sums[sl][:])

        # ========== phase B2: attn@V + output projection ====================
        with (
            tc.tile_pool(name="vload", bufs=6) as vload,
            tc.tile_pool(name="onorm", bufs=2) as onorm,
            tc.tile_pool(name="outst", bufs=2) as outst,
            tc.tile_pool(name="ppO", bufs=1, space="PSUM") as ppO,
            tc.tile_pool(name="ppW", bufs=2, space="PSUM") as ppW,
            tc.tile_pool(name="ppR2", bufs=2, space="PSUM") as ppR2,
        ):
            for sl in range(2):
                fq = _frames_qw(sl)
                psum_o = [
                    ppO.tile([P, 512], F32, tag=f"o{cc}", name=f"psum_o{sl}{cc}")
                    for cc in range(CC)
                ]
                rcB_ps = ppR2.tile([P, 512], F32, tag="rcb", name=f"rcB_ps{sl}")
                nc.tensor.matmul(rcB_ps[:], lhsT=ones_row[:], rhs=rc[sl][:])
                rcB = onorm.tile([P, 512], F32, tag="rcbs", name=f"rcB{sl}")
                nc.vector.tensor_copy(out=rcB[:], in_=rcB_ps[:])
                npair = len(fq) * NP2
                n = 0
                for f, qlo, qw in fq:
                    for t2 in range(NP2):
                        vt2 = vload.tile([P, 2, C], FP8)
                        nc.sync.dma_start(
                            vt2[:],
                            agv_out[f, t2 * 256 : (t2 + 1) * 256, :].rearrange(
                                "(j p) c -> p j c", p=P
                            ),
                        )
                        idx = sl * 16 + f * NP2 + t2
                        for cc in range(CC):
                            nc.tensor.matmul(
                                psum_o[cc][:, qlo:],
                                lhsT=vt2[:, :, cc * P : (cc + 1) * P],
                                rhs=at_sb[:, idx, :, :qw],
                                start=(n == 0),
                                stop=(n == npair - 1),
                                perf_mode=DR,
                            )
                        n += 1
                onrm = onorm.tile([P, CC, 512], FP8)
                for cc in range(CC):
                    nc.scalar.activation(
                        onrm[:, cc, :], psum_o[cc][:], Act.Identity,
                        scale=1.0 / 16.0,
                    )
                # output projection + 1/sum + bias + residual
                for oc in range(CC):
                    psw = ppW.tile([P, 512], F32, tag="sc", name="psw")
                    for g in range(2):
                        nc.tensor.matmul(
                            psw[:],
                            lhsT=wo_sb[:, 2 * g : 2 * g + 2, oc * P : (oc + 1) * P],
                            rhs=onrm[:, 2 * g : 2 * g + 2, :],
                            start=(g == 0),
                            stop=(g == 1),
                            perf_mode=DR,
                        )
                    o_sc = outst.tile([P, 512], F32, name="o_sc")
                    nc.vector.scalar_tensor_tensor(
                        out=o_sc[:],
                        in0=psw[:],
                        scalar=1.0,
                        in1=rcB[:],
                        op0=Alu.mult,
                        op1=Alu.mult,
                    )
                    o_out = outst.tile([P, 512], F32)
                    nc.vector.scalar_tensor_tensor(
                        out=o_out[:],
                        in0=o_sc[:],
                        scalar=boCol[:, oc : oc + 1],
                        in1=xo_sb[:, oc, sl * 512 : (sl + 1) * 512],
                        op0=Alu.add,
                        op1=Alu.add,
                    )
                    nc.sync.dma_start(
                        out[oc * P : (oc + 1) * P, sl * 512 : (sl + 1) * 512], o_out[:]
                    )


def kernel(x, gamma, wq, bq, wk, bk, wv, bv, wo, bo):
    global LAST_RESULTS
    _ensure_axon_hooks()
    x = np.asarray(x, dtype=np.float32)
    gamma = np.asarray(gamma, dtype=np.float32).reshape(C)
    f8 = ml_dtypes.float8_e4m3
    # x16 scale keeps the sigma=0.02 weights out of fp8's subnormal range
    # (unscaled on-device at each PSUM->SBUF copy); gamma is folded into the
    # input-channel rows of the q/k/v weights (h = normalize(x)*sqrt(C) only).
    g = np.asarray(gamma, np.float32).reshape(C, 1)
    ws = {
        "wqT": np.ascontiguousarray(np.asarray(wq, np.float32).T * g * 16.0).astype(f8),
        "wkT": np.ascontiguousarray(np.asarray(wk, np.float32).T * g * 16.0).astype(f8),
        "wvT": np.ascontiguousarray(np.asarray(wv, np.float32).T * g * 16.0).astype(f8),
        "woT": np.ascontiguousarray(np.asarray(wo, np.float32).T * 16.0).astype(f8),
    }
    bs = {
        "bq": np.asarray(bq, np.float32).reshape(C),
        "bk": np.asarray(bk, np.float32).reshape(C),
        "bv": np.asarray(bv, np.float32).reshape(C),
        "bo": np.asarray(bo, np.float32).reshape(C),
    }

    xs = x.reshape(C, SEQ)  # [c, f*h*w], frame = s // 1024
    xsub = xs.reshape(C, NF, NHW // P, P)  # [c, frame, subchunk, 128]

    in_maps = []
    for i in range(NCORES):
        xo_i = np.ascontiguousarray(xsub[:, :, i, :]).reshape(C, NHW)
        xfi_i = np.ascontiguousarray(xs[:, i * NHW : (i + 1) * NHW]).astype(
            ml_dtypes.bfloat16
        )
        in_maps.append(
            {"xfi": xfi_i, "xo": xo_i, "gamma": gamma, **ws, **bs}
        )

    nc = _build_nc()
    nc.finalize()  # run Bacc passes (multi-wait splitting etc.) before lowering
    res = run_bass_kernel_spmd(nc, in_maps, list(range(NCORES)))
    LAST_RESULTS = res

    out_full = np.empty((C, SEQ), np.float32)
    ov = out_full.reshape(C, NF, NHW // P, P)
    for i in range(NCORES):
        ov[:, :, i, :] = res.results[i]["out"].reshape(C, NF, P)
    return out_full.reshape(B, C, NF, H, W)
